# revision 1
# baseline (speedup 1.0000x reference)
"""AdaptiveCategoryMSA Trainium2 kernel (8 NeuronCores, data-parallel).

Host: category argmax + stable argsort; gather, logit-scale (folded into q)
and the per-head ones-column on V are fused into the shard step.
Device (per core = one batch-half of 8192 tokens = 64 groups of 128), per
head-pair streamed: S = qT.T @ kT (f32), rowmax via segmented reduce
(negate=True feeds the exp bias directly), exp -> E (bf16), PE-transpose
E -> ET, Y+rowsum = ET.T @ [V|1] (N=33; ones column gives the softmax
denominator on the PE), per-head reciprocal + normalization folded into the
Y psum->sbuf copy, Y -> YT via 2 PE transposes, out = YT.T @ Wt (+bias via
tensor_tensor add), bf16 out DMA. Post-softmax path is bf16; logits f32.
Sharding: core cx = 2*b + half handles batch b, tokens [8192*half, ...)
(in sorted order; groups never cross the half boundary since 8192 = 64*128).
"""
import sys
sys.path.insert(0, "/opt/trn_rl_repo")
import numpy as np

import concourse.bass as bass
import concourse.bacc as bacc
import concourse.mybir as mybir
from concourse.tile import TileContext
from concourse.bass_utils import run_bass_kernel_spmd

F32 = mybir.dt.float32
BF16 = mybir.dt.bfloat16

NUM_HEADS = 8
GS = 128          # group size (tokens per attention group)
NG_CORE = 64      # groups per core (8192 tokens)
C = 256           # channels
D = 32            # head dim

_cache = {}
_last_in_maps = None
PROFILE = False
LAST_EXEC_NS = None
LAST_TRACE = None


def _build(scale: float):
    nc = bacc.Bacc(
        "TRN2", target_bir_lowering=False, debug=False,
        enable_asserts=True, num_devices=8,
    )
    qkt = nc.dram_tensor("qkt", [NG_CORE, 128, 4, 128], F32, kind="ExternalInput")
    vg = nc.dram_tensor("vg", [NG_CORE, 128, 264], BF16, kind="ExternalInput")
    wt = nc.dram_tensor("wt", [2, 128, C], BF16, kind="ExternalInput")   # proj_w.T chunks
    bfull = nc.dram_tensor("bfull", [128, C], F32, kind="ExternalInput")
    idm = nc.dram_tensor("idm", [128, 128], BF16, kind="ExternalInput")
    out = nc.dram_tensor("out", [NG_CORE, 128, C], BF16, kind="ExternalOutput")

    with TileContext(nc) as tc:
        with tc.tile_pool(name="const", bufs=1) as cpool, \
             tc.tile_pool(name="sb", bufs=6) as sb, \
             tc.tile_pool(name="psS", bufs=2, space="PSUM") as psS, \
             tc.tile_pool(name="psT", bufs=2, space="PSUM") as psT, \
             tc.tile_pool(name="psY", bufs=2, space="PSUM") as psY:
            psO = psY
            wt_sb = cpool.tile([128, 2 * C], BF16)
            nc.sync.dma_start(wt_sb[:, :].rearrange("p (c n) -> p c n", c=2), wt[:, :, :].rearrange("c p n -> p c n"))
            bias_sb = cpool.tile([128, C], F32)
            nc.sync.dma_start(bias_sb[:, :], bfull[:, :])
            idm_sb = cpool.tile([128, 128], BF16)
            nc.sync.dma_start(idm_sb[:, :], idm[:, :])

            def alloc_group(g):
                qk = sb.tile([128, 512], F32, tag="qk")
                nc.sync.dma_start(
                    qk[:, :].rearrange("p (c j) -> p c j", c=4),
                    qkt[g, :, :, :])
                vt = sb.tile([128, 264], BF16, tag="vt")
                nc.sync.dma_start(vt[:, :], vg[g, :, :])
                negmax = sb.tile([128, 8], F32, tag="negmax")
                rinv = sb.tile([128, 8], F32, tag="rinv")
                esb = sb.tile([128, 1024], BF16, tag="esb")
                ptsb = sb.tile([128, 1024], BF16, tag="ptsb")
                ysb = sb.tile([128, C], BF16, tag="ysb")
                return {"qk": qk, "vt": vt, "negmax": negmax, "rinv": rinv,
                        "esb": esb, "ptsb": ptsb, "ysb": ysb}

            def emit_pair(t, half, pair):
                qk, vt = t["qk"], t["vt"]
                negmax, rinv = t["negmax"], t["rinv"]
                esb, ptsb, ysb = t["esb"], t["ptsb"], t["ysb"]
                o0 = 4 * half + 2 * pair
                smega = psS.tile([128, 1024], F32, tag="smega")
                for k in range(2):
                    hm = 2 * pair + k
                    lhs = qk[32 * hm:32 * hm + 32, 128 * half:128 * half + 128]
                    rhs = qk[32 * hm:32 * hm + 32,
                             128 * (2 + half):128 * (2 + half) + 128]
                    tp = (96, 0) if hm == 3 else None
                    nc.tensor.matmul(smega[:, 512 * k:512 * k + 128],
                                     lhs, rhs, start=True, stop=True,
                                     tile_position=tp)
                with tc.high_priority(offset=260):
                    nc.vector.tensor_reduce(
                        negmax[:, o0:o0 + 2],
                        smega[:, :].rearrange("p (s j) -> p s j", s=2)[:, :, 0:128],
                        axis=mybir.AxisListType.X,
                        op=mybir.AluOpType.max, negate=True)
                for k in range(2):
                    h = o0 + k
                    with tc.high_priority(offset=60):
                        nc.scalar.activation(
                            esb[:, 128 * h:128 * h + 128],
                            smega[:, 512 * k:512 * k + 128],
                            mybir.ActivationFunctionType.Exp,
                            bias=negmax[:, h:h + 1], scale=1.0)
                for k in range(2):
                    h = o0 + k
                    ptp = psT.tile([128, 128], BF16, tag="ptp")
                    nc.tensor.transpose(ptp[:, :], esb[:, 128 * h:128 * h + 128],
                                        idm_sb[:, :])
                    if h in (0, 4):
                        nc.scalar.copy(ptsb[:, 128 * h:128 * h + 128], ptp[:, :])
                    else:
                        nc.vector.tensor_copy(ptsb[:, 128 * h:128 * h + 128],
                                              ptp[:, :])
                for k in range(2):
                    h = o0 + k
                    ypf = psY.tile([128, 256], F32, tag="yp")
                    yp = ypf[:, 0:33]
                    nc.tensor.matmul(yp[:, :], ptsb[:, 128 * h:128 * h + 128],
                                     vt[:, 33 * h:33 * h + 33],
                                     start=True, stop=True)
                    with tc.high_priority(offset=200):
                        nc.vector.reciprocal(rinv[:, h:h + 1], ypf[:, 32:33])
                    with tc.high_priority(offset=60):
                        if h % 2 == 0:
                            nc.vector.tensor_scalar_mul(
                                ysb[:, 32 * h:32 * h + 32], ypf[:, 0:32],
                                rinv[:, h:h + 1])
                        else:
                            nc.scalar.mul(ysb[:, 32 * h:32 * h + 32],
                                          ypf[:, 0:32], rinv[:, h:h + 1])

            def emit_tail(t, g):
                ysb = t["ysb"]
                ytsb = sb.tile([128, 256], BF16, tag="ytsb")
                for ck in range(2):
                    ytp = psT.tile([128, 128], BF16, tag="ptp")
                    nc.tensor.transpose(ytp[:, :], ysb[:, 128 * ck:128 * ck + 128],
                                        idm_sb[:, :])
                    nc.scalar.copy(ytsb[:, 128 * ck:128 * ck + 128], ytp[:, :])
                op = psO.tile([128, 256], F32, tag="yp")
                nc.tensor.matmul(op[:, :], ytsb[:, 0:128], wt_sb[:, 0:C],
                                 start=True, stop=False)
                nc.tensor.matmul(op[:, :], ytsb[:, 128:256], wt_sb[:, C:2 * C],
                                 start=False, stop=True)
                osb = sb.tile([128, C], BF16, tag="osb")
                nc.vector.tensor_add(osb[:, :], op[:, :], bias_sb[:, :])
                nc.sync.dma_start(out[g, :, :], osb[:, :])

            GRP = 4
            for gp in range(NG_CORE // GRP):
                gs = [GRP * gp + i for i in range(GRP)]
                ts = [alloc_group(g) for g in gs]
                for half in range(2):
                    for pair in range(2):
                        r = (2 * half + pair) % GRP
                        for t in ts[r:] + ts[:r]:
                            emit_pair(t, half, pair)
                for t, g in zip(ts, gs):
                    emit_tail(t, g)

    nc.finalize()
    return nc


def kernel(qkv, sim, proj_w, proj_b, logit_scale, h=128, w=128, **_unused):
    qkv = np.ascontiguousarray(np.asarray(qkv, dtype=np.float32))
    sim = np.asarray(sim, dtype=np.float32)
    proj_w = np.asarray(proj_w, dtype=np.float32)
    proj_b = np.asarray(proj_b, dtype=np.float32)
    ls = float(np.asarray(logit_scale, dtype=np.float32).reshape(-1)[0])
    scale = float(np.exp(min(ls, float(np.log(100.0)))))

    b, n, c3 = qkv.shape
    assert (b, n, c3) == (4, 16384, 768)

    tk = np.argmax(sim, axis=-1)                      # [b, n]
    sort_idx = np.argsort(tk, axis=-1, kind="stable")  # [b, n]

    if "g" not in _cache:
        _cache["g"] = _build(1.0)
    nc = _cache["g"]

    import ml_dtypes
    wt_full = np.ascontiguousarray(proj_w.T)                       # [c, o]
    wt_in = np.ascontiguousarray(wt_full.reshape(2, 128, 256)).astype(ml_dtypes.bfloat16)
    bfull = np.ascontiguousarray(np.broadcast_to(proj_b[None, :], (128, 256)))
    idm = np.eye(128, dtype=np.float32).astype(ml_dtypes.bfloat16)

    in_maps = []
    for cx in range(8):
        bi, half = cx // 2, cx % 2
        perm = sort_idx[bi, 8192 * half:8192 * (half + 1)]
        shuf = qkv[bi][perm]                                        # [8192, 768]
        qkpart = np.ascontiguousarray(shuf[:, 0:512])
        qkpart[:, 0:256] *= scale           # fold logit scale into q
        qkt = np.ascontiguousarray(
            qkpart.reshape(64, 128, 4, 128).transpose(0, 3, 2, 1))
        vga = np.empty((64, 128, 8, 33), dtype=np.float32)
        vga[:, :, :, 0:32] = shuf[:, 512:768].reshape(64, 128, 8, 32)
        vga[:, :, :, 32] = 1.0
        vgv = np.ascontiguousarray(vga.reshape(64, 128, 264)).astype(ml_dtypes.bfloat16)
        in_maps.append({"qkt": qkt, "vg": vgv, "wt": wt_in,
                        "bfull": bfull, "idm": idm})

    global LAST_EXEC_NS, LAST_TRACE
    if PROFILE:
        import tempfile
        td = tempfile.mkdtemp(prefix="msa_prof_")
        res = run_bass_kernel_spmd(nc, in_maps, core_ids=list(range(8)),
                                   trace=True, tmpdir=td)
        LAST_EXEC_NS = res.exec_time_ns
        LAST_TRACE = td
    else:
        global _last_in_maps
    _last_in_maps = in_maps
    res = run_bass_kernel_spmd(nc, in_maps, core_ids=list(range(8)))

    outf = np.empty((4, 16384, 256), dtype=np.float32)
    for cx in range(8):
        bi, half = cx // 2, cx % 2
        perm = sort_idx[bi, 8192 * half:8192 * (half + 1)]
        y = np.asarray(res.results[cx]["out"]).astype(np.float32).reshape(8192, 256)
        outf[bi][perm] = y
    return outf



def bench_exec(in_maps, nc, iters=8):
    """Time repeated NEFF executions with device-resident inputs.

    Returns (min_s, all_s). Mirrors bass2jax.run_bass_via_pjrt's multi-core
    path but keeps the jitted callable and input device arrays across calls.
    """
    import time as _time
    import jax
    from jax.sharding import Mesh, PartitionSpec, NamedSharding
    from jax.experimental.shard_map import shard_map
    from concourse import bass2jax, mybir as mb

    bass2jax.install_neuronx_cc_hook()
    n_cores = len(in_maps)
    partition_name = nc.partition_id_tensor.name if nc.partition_id_tensor else None
    in_names, out_names, out_avals, zero_outs = [], [], [], []
    for alloc in nc.m.functions[0].allocations:
        if not isinstance(alloc, mb.MemoryLocationSet):
            continue
        name = alloc.memorylocations[0].name
        if alloc.kind == "ExternalInput":
            if name != partition_name:
                in_names.append(name)
        elif alloc.kind == "ExternalOutput":
            shape = tuple(alloc.tensor_shape)
            dtype = mb.dt.np(alloc.dtype)
            out_names.append(name)
            out_avals.append(jax.core.ShapedArray(shape, dtype))
            zero_outs.append(np.zeros(shape, dtype))
    n_params = len(in_names)
    n_outs = len(out_avals)
    all_in_names = list(in_names) + list(out_names)
    if partition_name is not None:
        all_in_names.append(partition_name)

    def _body(*args):
        operands = list(args)
        if partition_name is not None:
            operands.append(bass2jax.partition_id_tensor())
        outs = bass2jax._bass_exec_p.bind(
            *operands,
            out_avals=tuple(out_avals),
            in_names=tuple(all_in_names),
            out_names=tuple(out_names),
            lowering_input_output_aliases=(),
            sim_require_finite=True,
            sim_require_nnan=True,
            nc=nc,
        )
        return tuple(outs)

    devices = jax.devices()[:n_cores]
    mesh = Mesh(np.asarray(devices), ("core",))
    pspec = PartitionSpec("core")
    sharded = jax.jit(
        shard_map(_body, mesh=mesh, in_specs=(pspec,) * (n_params + n_outs),
                  out_specs=(pspec,) * n_outs, check_rep=False),
        donate_argnums=tuple(range(n_params, n_params + n_outs)),
        keep_unused=True,
    )
    shard = NamedSharding(mesh, pspec)
    concat_in = [
        jax.device_put(
            np.concatenate([np.asarray(in_maps[c][nm]) for c in range(n_cores)], axis=0),
            shard)
        for nm in in_names
    ]
    jax.block_until_ready(concat_in)

    def zeros():
        zs = [jax.device_put(np.zeros((n_cores * z.shape[0], *z.shape[1:]), z.dtype),
                             shard) for z in zero_outs]
        jax.block_until_ready(zs)
        return zs

    times = []
    out = sharded(*concat_in, *zeros())  # warm (compile)
    jax.block_until_ready(out)
    for _ in range(iters):
        zs = zeros()
        t0 = _time.perf_counter()
        out = sharded(*concat_in, *zs)
        jax.block_until_ready(out)
        times.append(_time.perf_counter() - t0)
    return min(times), times


if __name__ == "__main__":
    rng = np.random.default_rng(0)
    qkv = rng.standard_normal((4, 16384, 768), dtype=np.float32)
    sim = rng.standard_normal((4, 16384, 64), dtype=np.float32)
    pw = (rng.standard_normal((256, 256), dtype=np.float32) * 0.02)
    pb = np.zeros(256, dtype=np.float32)
    lsc = np.log(10.0 * np.ones((1, 1), dtype=np.float32))
    o = kernel(qkv=qkv, sim=sim, proj_w=pw, proj_b=pb, logit_scale=lsc)
    print("ran", o.shape, o.dtype)



# revision 4
# speedup vs baseline: 1.3522x; 1.3522x over previous
"""AdaptiveCategoryMSA Trainium2 kernel (8 NeuronCores, data-parallel).

Host: category argmax + stable argsort; gather + logit-scale fold + fp16
pack. Device (per core = one batch-half = 64 groups of 128 tokens):
fp16 S matmuls (f32 psum), DVE rowmax (negate -> exp bias), 8 per-head
biased Act exps -> E fp16 sbuf, 8 PE transposes -> ET psum fp16, one DVE
mega-copy -> ET sbuf, 8 Y matmuls with ones-column denominators, DVE
reciprocal, Y normalize, 2 YT transposes + copy, 2 proj matmuls, Act
outcopy, batched DMAs (4 groups per DMA). Software-pipelined with a
1-group lag so all five engines stream.
Sharding: core cx = 2*b + half handles batch b, tokens [8192*half, ...).
"""
import sys
sys.path.insert(0, "/opt/trn_rl_repo")
import numpy as np

import concourse.bass as bass
import concourse.bacc as bacc
import concourse.mybir as mybir
from concourse.tile import TileContext
from concourse.bass_utils import run_bass_kernel_spmd

F32 = mybir.dt.float32
F16 = mybir.dt.float16

NG = 64          # groups per core
GB = 4           # groups per DMA batch
W = 776          # per-group input width: qk 512 + v(+ones) 264
C = 256

_cache = {}
_last_in_maps = None

# op placement toggles (tuned against the timeline cost model)
YNORM_BCAST = True      # one DVE tensor_tensor with broadcast rinv
YTCOPY_ENGINE = "pool"  # pool | vector | scalar
OUTCOPY_ENGINE = "scalar"


def _eng(nc, name):
    return {"pool": nc.gpsimd, "vector": nc.vector, "scalar": nc.scalar}[name]


def _build(with_bias: bool):
    nc = bacc.Bacc(
        "TRN2", target_bir_lowering=False, debug=False,
        enable_asserts=True, num_devices=8,
    )
    qvd = nc.dram_tensor("qvd", [NG // GB, GB, 128, W], F16, kind="ExternalInput")
    wtd = nc.dram_tensor("wtd", [128, 2 * C], F16, kind="ExternalInput")
    idmd = nc.dram_tensor("idmd", [128, 128], F16, kind="ExternalInput")
    if with_bias:
        biasd = nc.dram_tensor("biasd", [1, C], F16, kind="ExternalInput")
    outd = nc.dram_tensor("outd", [NG // GB, GB, 128, C], F16, kind="ExternalOutput")

    AX = mybir.AxisListType.X
    EXP = mybir.ActivationFunctionType.Exp

    with TileContext(nc) as tc:
        with tc.tile_pool(name="const", bufs=1) as cpool, \
             tc.tile_pool(name="sbin", bufs=2) as sbin, \
             tc.tile_pool(name="sbe", bufs=2) as sbe, \
             tc.tile_pool(name="sbsm", bufs=2) as sbsm, \
             tc.tile_pool(name="sbo", bufs=2) as sbo, \
             tc.tile_pool(name="psS", bufs=2, space="PSUM") as psS, \
             tc.tile_pool(name="psET", bufs=1, space="PSUM") as psET, \
             tc.tile_pool(name="psY", bufs=1, space="PSUM") as psY, \
             tc.tile_pool(name="psYT", bufs=1, space="PSUM") as psYT, \
             tc.tile_pool(name="psO", bufs=1, space="PSUM") as psO:

            wt_sb = cpool.tile([128, 2 * C], F16)
            nc.sync.dma_start(wt_sb[:, :], wtd[:, :])
            idm = cpool.tile([128, 128], F16)
            nc.sync.dma_start(idm[:, :], idmd[:, :])
            if with_bias:
                bias_sb = cpool.tile([1, C], F16)
                nc.sync.dma_start(bias_sb[:, :], biasd[:, :])
                ones_sb = cpool.tile([1, 128], F16)
                nc.gpsimd.memset(ones_sb[:, :], 1.0)

            shared = {}

            def head(g):
                if g % GB == 0:
                    qv = sbin.tile([128, GB * W], F16, tag="qv")
                    nc.sync.dma_start(
                        qv.rearrange("p (g j) -> p g j", g=GB),
                        qvd[g // GB, :, :, :].rearrange("g p j -> p g j"))
                    shared["qv"] = qv
                qv = shared["qv"]
                off = (g % GB) * W
                smega = psS.tile([128, 1024], F32, tag="s")
                for h in range(8):
                    c, hm = h // 4, h % 4
                    lhs = qv[32 * hm:32 * hm + 32, off + 128 * c: off + 128 * c + 128]
                    rhs = qv[32 * hm:32 * hm + 32,
                             off + 256 + 128 * c: off + 256 + 128 * c + 128]
                    tp = (96, 0) if hm == 3 else None
                    nc.tensor.matmul(smega[:, 128 * h:128 * h + 128], lhs, rhs,
                                     start=True, stop=True, tile_position=tp)
                negm = sbsm.tile([128, 8], F32, tag="negm")
                nc.vector.tensor_reduce(
                    negm[:, :], smega.rearrange("p (s j) -> p s j", s=8),
                    axis=AX, op=mybir.AluOpType.max, negate=True)
                esb = sbe.tile([128, 1024], F16, tag="esb")
                for h in range(8):
                    nc.scalar.activation(
                        esb[:, 128 * h:128 * h + 128],
                        smega[:, 128 * h:128 * h + 128],
                        EXP, bias=negm[:, h:h + 1], scale=1.0)
                return {"qv": qv, "off": off, "esb": esb, "g": g}

            def tail(st):
                qv, off, esb, g = st["qv"], st["off"], st["esb"], st["g"]
                etp = psET.tile([128, 1024], F16, tag="et")
                for h in range(8):
                    nc.tensor.transpose(etp[:, 128 * h:128 * h + 128],
                                        esb[:, 128 * h:128 * h + 128], idm[:, :])
                etsb = sbe.tile([128, 1024], F16, tag="etsb")
                nc.vector.tensor_copy(etsb[:, :], etp[:, :])
                yp = psY.tile([128, 264], F32, tag="y")
                for h in range(8):
                    nc.tensor.matmul(
                        yp[:, 33 * h:33 * h + 33],
                        etsb[:, 128 * h:128 * h + 128],
                        qv[:, off + 512 + 33 * h: off + 512 + 33 * h + 33],
                        start=True, stop=True)
                rinv = sbsm.tile([128, 8], F32, tag="rinv")
                y3 = yp.rearrange("p (h j) -> p h j", h=8)
                nc.vector.reciprocal(rinv.rearrange("p (h j) -> p h j", j=1),
                                     y3[:, :, 32:33])
                ysb = sbsm.tile([128, 256], F16, tag="ysb")
                if YNORM_BCAST:
                    nc.vector.tensor_tensor(
                        ysb.rearrange("p (h j) -> p h j", h=8),
                        y3[:, :, 0:32],
                        rinv.rearrange("p (h j) -> p h j", j=1).broadcast_to([128, 8, 32]),
                        op=mybir.AluOpType.mult)
                else:
                    for h in range(8):
                        eng = nc.vector if h % 2 == 0 else nc.gpsimd
                        eng.tensor_scalar_mul(
                            ysb[:, 32 * h:32 * h + 32],
                            yp[:, 33 * h:33 * h + 32],
                            rinv[:, h:h + 1])
                ytp = psYT.tile([128, 256], F16, tag="yt")
                for ck in range(2):
                    nc.tensor.transpose(ytp[:, 128 * ck:128 * ck + 128],
                                        ysb[:, 128 * ck:128 * ck + 128], idm[:, :])
                ytsb = sbsm.tile([128, 256], F16, tag="ytsb")
                if YTCOPY_ENGINE == "pool":
                    nc.gpsimd.tensor_copy(ytsb[:, :], ytp[:, :])
                elif YTCOPY_ENGINE == "vector":
                    nc.vector.tensor_copy(ytsb[:, :], ytp[:, :])
                else:
                    nc.scalar.copy(ytsb[:, :], ytp[:, :])
                op = psO.tile([128, C], F32, tag="o")
                if with_bias:
                    nc.tensor.matmul(op[:, :], ones_sb[:, :], bias_sb[:, :],
                                     start=True, stop=False)
                nc.tensor.matmul(op[:, :], ytsb[:, 0:128], wt_sb[:, 0:C],
                                 start=not with_bias, stop=False)
                nc.tensor.matmul(op[:, :], ytsb[:, 128:256], wt_sb[:, C:2 * C],
                                 start=False, stop=True)
                if g % GB == 0:
                    shared["osb"] = sbo.tile([128, GB * C], F16, tag="osb", name="osb")
                osb = shared["osb"]
                gi = g % GB
                if OUTCOPY_ENGINE == "scalar":
                    nc.scalar.copy(osb[:, C * gi:C * gi + C], op[:, :])
                elif OUTCOPY_ENGINE == "vector":
                    nc.vector.tensor_copy(osb[:, C * gi:C * gi + C], op[:, :])
                else:
                    nc.gpsimd.tensor_copy(osb[:, C * gi:C * gi + C], op[:, :])
                if gi == GB - 1:
                    nc.sync.dma_start(
                        outd[g // GB, :, :, :].rearrange("g p j -> p g j"),
                        osb.rearrange("p (g j) -> p g j", g=GB))

            prev = None
            for g in range(NG):
                st = head(g)
                if prev is not None:
                    tail(prev)
                prev = st
            tail(prev)

    nc.finalize()
    return nc


def _prep_inputs(qkv, sim, proj_w, proj_b, scale):
    """Host-side shard + pack. Returns (in_maps, sort_indices, with_bias)."""
    b, n, _ = qkv.shape
    tk = np.argmax(sim, axis=-1)
    sort_idx = np.argsort(tk, axis=-1, kind="stable")

    wt_full = np.ascontiguousarray(proj_w.T).astype(np.float16)   # [cin, cout]
    with_bias = bool(np.any(proj_b != 0))
    bias16 = proj_b.reshape(1, C).astype(np.float16)
    idm = np.eye(128, dtype=np.float16)

    in_maps = []
    for cx in range(8):
        bi, half = cx // 2, cx % 2
        perm = sort_idx[bi, 8192 * half:8192 * (half + 1)]
        shuf = qkv[bi][perm].astype(np.float32)                    # [8192, 768]
        qk = shuf[:, 0:512].copy()
        qk[:, 0:256] *= scale
        # [g, tok, 4, 128] -> [g, p=ch, c, tok]
        qkt = qk.astype(np.float16).reshape(NG, 128, 4, 128).transpose(0, 3, 2, 1)
        qkt = qkt.reshape(NG, 128, 512)
        vpart = np.empty((NG, 128, 8, 33), dtype=np.float16)
        vpart[:, :, :, 0:32] = shuf[:, 512:768].reshape(NG, 128, 8, 32)
        vpart[:, :, :, 32] = 1.0
        qv = np.concatenate([qkt, vpart.reshape(NG, 128, 264)], axis=2)
        qv = np.ascontiguousarray(qv.reshape(NG // GB, GB, 128, W))
        m = {"qvd": qv, "wtd": wt_full, "idmd": idm}
        if with_bias:
            m["biasd"] = bias16
        in_maps.append(m)
    return in_maps, sort_idx, with_bias


def kernel(qkv, sim, proj_w, proj_b, logit_scale, h=128, w=128, **_unused):
    qkv = np.ascontiguousarray(np.asarray(qkv, dtype=np.float32))
    sim = np.asarray(sim, dtype=np.float32)
    proj_w = np.asarray(proj_w, dtype=np.float32)
    proj_b = np.asarray(proj_b, dtype=np.float32)
    ls = float(np.asarray(logit_scale, dtype=np.float32).reshape(-1)[0])
    scale = float(np.exp(min(ls, float(np.log(100.0)))))

    b, n, c3 = qkv.shape
    assert (b, n, c3) == (4, 16384, 768)

    in_maps, sort_idx, with_bias = _prep_inputs(qkv, sim, proj_w, proj_b, scale)

    key = ("b" if with_bias else "nb")
    if key not in _cache:
        _cache[key] = _build(with_bias)
    nc = _cache[key]

    global _last_in_maps
    _last_in_maps = in_maps
    res = run_bass_kernel_spmd(nc, in_maps, core_ids=list(range(8)))

    outf = np.empty((4, 16384, 256), dtype=np.float32)
    for cx in range(8):
        bi, half = cx // 2, cx % 2
        perm = sort_idx[bi, 8192 * half:8192 * (half + 1)]
        y = np.asarray(res.results[cx]["outd"]).astype(np.float32).reshape(8192, 256)
        outf[bi][perm] = y
    return outf


if __name__ == "__main__":
    rng = np.random.default_rng(0)
    qkv = rng.standard_normal((4, 16384, 768), dtype=np.float32)
    sim = rng.standard_normal((4, 16384, 64), dtype=np.float32)
    pw = (rng.standard_normal((256, 256), dtype=np.float32) * 0.02)
    pb = np.zeros(256, dtype=np.float32)
    lsc = np.log(10.0 * np.ones((1, 1), dtype=np.float32))
    o = kernel(qkv=qkv, sim=sim, proj_w=pw, proj_b=pb, logit_scale=lsc)
    print("ran", o.shape, o.dtype)


# revision 5
# speedup vs baseline: 1.4254x; 1.0541x over previous
"""AdaptiveCategoryMSA Trainium2 kernel (8 NeuronCores, data-parallel).

Host: category argmax + stable argsort; gather + logit-scale fold + fp16
pack. Device (per core = one batch-half = 64 groups of 128 tokens):
fp16 S matmuls (f32 psum), DVE rowmax (negate -> exp bias), 8 per-head
biased Act exps -> E fp16 sbuf, 8 PE transposes -> ET psum fp16, one DVE
mega-copy -> ET sbuf, 8 Y matmuls with ones-column denominators, DVE
reciprocal, Y normalize, 2 YT transposes + copy, 2 proj matmuls, Act
outcopy, batched DMAs (4 groups per DMA). Software-pipelined with a
1-group lag so all five engines stream.
Sharding: core cx = 2*b + half handles batch b, tokens [8192*half, ...).
"""
import sys
sys.path.insert(0, "/opt/trn_rl_repo")
import numpy as np

import concourse.bass as bass
import concourse.bacc as bacc
import concourse.mybir as mybir
from concourse.tile import TileContext
from concourse.bass_utils import run_bass_kernel_spmd

F32 = mybir.dt.float32
F16 = mybir.dt.float16

NG = 64          # groups per core
GB = 4           # groups per DMA batch
W = 776          # per-group input width: qk 512 + v(+ones) 264
C = 256

_cache = {}
_last_in_maps = None

# op placement toggles (tuned against the timeline cost model)
YNORM_BCAST = True      # one DVE tensor_tensor with broadcast rinv
YTCOPY_ENGINE = "pool"  # pool | vector | scalar
OUTCOPY_ENGINE = "pool"


def _eng(nc, name):
    return {"pool": nc.gpsimd, "vector": nc.vector, "scalar": nc.scalar}[name]


def _build(with_bias: bool):
    nc = bacc.Bacc(
        "TRN2", target_bir_lowering=False, debug=False,
        enable_asserts=True, num_devices=8,
    )
    qvd = nc.dram_tensor("qvd", [NG // GB, GB, 128, W], F16, kind="ExternalInput")
    wtd = nc.dram_tensor("wtd", [128, 2 * C], F16, kind="ExternalInput")
    idmd = nc.dram_tensor("idmd", [128, 128], F16, kind="ExternalInput")
    if with_bias:
        biasd = nc.dram_tensor("biasd", [1, C], F16, kind="ExternalInput")
    outd = nc.dram_tensor("outd", [NG // GB, GB, 128, C], F16, kind="ExternalOutput")

    AX = mybir.AxisListType.X
    EXP = mybir.ActivationFunctionType.Exp

    with TileContext(nc) as tc:
        with tc.tile_pool(name="const", bufs=1) as cpool, \
             tc.tile_pool(name="sbin", bufs=2) as sbin, \
             tc.tile_pool(name="sbe", bufs=2) as sbe, \
             tc.tile_pool(name="sbsm", bufs=2) as sbsm, \
             tc.tile_pool(name="sbo", bufs=2) as sbo, \
             tc.tile_pool(name="psS", bufs=2, space="PSUM") as psS, \
             tc.tile_pool(name="psET", bufs=1, space="PSUM") as psET, \
             tc.tile_pool(name="psY", bufs=1, space="PSUM") as psY, \
             tc.tile_pool(name="psYT", bufs=1, space="PSUM") as psYT, \
             tc.tile_pool(name="psO", bufs=1, space="PSUM") as psO:

            wt_sb = cpool.tile([128, 2 * C], F16)
            nc.sync.dma_start(wt_sb[:, :], wtd[:, :])
            idm = cpool.tile([128, 128], F16)
            nc.sync.dma_start(idm[:, :], idmd[:, :])
            if with_bias:
                bias_sb = cpool.tile([1, C], F16)
                nc.sync.dma_start(bias_sb[:, :], biasd[:, :])
                ones_sb = cpool.tile([1, 128], F16)
                nc.gpsimd.memset(ones_sb[:, :], 1.0)

            shared = {}

            def head(g):
                if g % GB == 0:
                    qv = sbin.tile([128, GB * W], F16, tag="qv")
                    nc.sync.dma_start(
                        qv.rearrange("p (g j) -> p g j", g=GB),
                        qvd[g // GB, :, :, :].rearrange("g p j -> p g j"))
                    shared["qv"] = qv
                qv = shared["qv"]
                off = (g % GB) * W
                smega = psS.tile([128, 1024], F32, tag="s")
                for h in range(8):
                    c, hm = h // 4, h % 4
                    lhs = qv[32 * hm:32 * hm + 32, off + 128 * c: off + 128 * c + 128]
                    rhs = qv[32 * hm:32 * hm + 32,
                             off + 256 + 128 * c: off + 256 + 128 * c + 128]
                    tp = (96, 0) if hm == 3 else None
                    nc.tensor.matmul(smega[:, 128 * h:128 * h + 128], lhs, rhs,
                                     start=True, stop=True, tile_position=tp)
                negm = sbsm.tile([128, 8], F32, tag="negm")
                nc.vector.tensor_reduce(
                    negm[:, :], smega.rearrange("p (s j) -> p s j", s=8),
                    axis=AX, op=mybir.AluOpType.max, negate=True)
                esb = sbe.tile([128, 1024], F16, tag="esb")
                for h in range(8):
                    nc.scalar.activation(
                        esb[:, 128 * h:128 * h + 128],
                        smega[:, 128 * h:128 * h + 128],
                        EXP, bias=negm[:, h:h + 1], scale=1.0)
                return {"qv": qv, "off": off, "esb": esb, "g": g}

            def tail(st):
                qv, off, esb, g = st["qv"], st["off"], st["esb"], st["g"]
                etp = psET.tile([128, 1024], F16, tag="et")
                for h in range(8):
                    nc.tensor.transpose(etp[:, 128 * h:128 * h + 128],
                                        esb[:, 128 * h:128 * h + 128], idm[:, :])
                etsb = sbe.tile([128, 1024], F16, tag="etsb")
                nc.vector.tensor_copy(etsb[:, :], etp[:, :])
                yp = psY.tile([128, 264], F32, tag="y")
                for h in range(8):
                    nc.tensor.matmul(
                        yp[:, 33 * h:33 * h + 33],
                        etsb[:, 128 * h:128 * h + 128],
                        qv[:, off + 512 + 33 * h: off + 512 + 33 * h + 33],
                        start=True, stop=True)
                rinv = sbsm.tile([128, 8], F32, tag="rinv")
                y3 = yp.rearrange("p (h j) -> p h j", h=8)
                nc.vector.reciprocal(rinv.rearrange("p (h j) -> p h j", j=1),
                                     y3[:, :, 32:33])
                ysb = sbsm.tile([128, 256], F16, tag="ysb")
                if YNORM_BCAST:
                    nc.vector.tensor_tensor(
                        ysb.rearrange("p (h j) -> p h j", h=8),
                        y3[:, :, 0:32],
                        rinv.rearrange("p (h j) -> p h j", j=1).broadcast_to([128, 8, 32]),
                        op=mybir.AluOpType.mult)
                else:
                    for h in range(8):
                        eng = nc.vector if h % 2 == 0 else nc.gpsimd
                        eng.tensor_scalar_mul(
                            ysb[:, 32 * h:32 * h + 32],
                            yp[:, 33 * h:33 * h + 32],
                            rinv[:, h:h + 1])
                ytp = psYT.tile([128, 256], F16, tag="yt")
                for ck in range(2):
                    nc.tensor.transpose(ytp[:, 128 * ck:128 * ck + 128],
                                        ysb[:, 128 * ck:128 * ck + 128], idm[:, :])
                ytsb = sbsm.tile([128, 256], F16, tag="ytsb")
                if YTCOPY_ENGINE == "pool":
                    nc.gpsimd.tensor_copy(ytsb[:, :], ytp[:, :])
                elif YTCOPY_ENGINE == "vector":
                    nc.vector.tensor_copy(ytsb[:, :], ytp[:, :])
                else:
                    nc.scalar.copy(ytsb[:, :], ytp[:, :])
                op = psO.tile([128, C], F32, tag="o")
                if with_bias:
                    nc.tensor.matmul(op[:, :], ones_sb[:, :], bias_sb[:, :],
                                     start=True, stop=False)
                nc.tensor.matmul(op[:, :], ytsb[:, 0:128], wt_sb[:, 0:C],
                                 start=not with_bias, stop=False)
                nc.tensor.matmul(op[:, :], ytsb[:, 128:256], wt_sb[:, C:2 * C],
                                 start=False, stop=True)
                if g % GB == 0:
                    shared["osb"] = sbo.tile([128, GB * C], F16, tag="osb", name="osb")
                osb = shared["osb"]
                gi = g % GB
                if OUTCOPY_ENGINE == "scalar":
                    nc.scalar.copy(osb[:, C * gi:C * gi + C], op[:, :])
                elif OUTCOPY_ENGINE == "vector":
                    nc.vector.tensor_copy(osb[:, C * gi:C * gi + C], op[:, :])
                else:
                    nc.gpsimd.tensor_copy(osb[:, C * gi:C * gi + C], op[:, :])
                if gi == GB - 1:
                    nc.sync.dma_start(
                        outd[g // GB, :, :, :].rearrange("g p j -> p g j"),
                        osb.rearrange("p (g j) -> p g j", g=GB))

            prev = None
            for g in range(NG):
                st = head(g)
                if prev is not None:
                    tail(prev)
                prev = st
            tail(prev)

    nc.finalize()
    return nc


def _prep_inputs(qkv, sim, proj_w, proj_b, scale):
    """Host-side shard + pack. Returns (in_maps, sort_indices, with_bias)."""
    b, n, _ = qkv.shape
    tk = np.argmax(sim, axis=-1)
    sort_idx = np.argsort(tk, axis=-1, kind="stable")

    wt_full = np.ascontiguousarray(proj_w.T).astype(np.float16)   # [cin, cout]
    with_bias = bool(np.any(proj_b != 0))
    bias16 = proj_b.reshape(1, C).astype(np.float16)
    idm = np.eye(128, dtype=np.float16)

    in_maps = []
    for cx in range(8):
        bi, half = cx // 2, cx % 2
        perm = sort_idx[bi, 8192 * half:8192 * (half + 1)]
        shuf = qkv[bi][perm].astype(np.float32)                    # [8192, 768]
        qk = shuf[:, 0:512].copy()
        qk[:, 0:256] *= scale
        # [g, tok, 4, 128] -> [g, p=ch, c, tok]
        qkt = qk.astype(np.float16).reshape(NG, 128, 4, 128).transpose(0, 3, 2, 1)
        qkt = qkt.reshape(NG, 128, 512)
        vpart = np.empty((NG, 128, 8, 33), dtype=np.float16)
        vpart[:, :, :, 0:32] = shuf[:, 512:768].reshape(NG, 128, 8, 32)
        vpart[:, :, :, 32] = 1.0
        qv = np.concatenate([qkt, vpart.reshape(NG, 128, 264)], axis=2)
        qv = np.ascontiguousarray(qv.reshape(NG // GB, GB, 128, W))
        m = {"qvd": qv, "wtd": wt_full, "idmd": idm}
        if with_bias:
            m["biasd"] = bias16
        in_maps.append(m)
    return in_maps, sort_idx, with_bias


def kernel(qkv, sim, proj_w, proj_b, logit_scale, h=128, w=128, **_unused):
    qkv = np.ascontiguousarray(np.asarray(qkv, dtype=np.float32))
    sim = np.asarray(sim, dtype=np.float32)
    proj_w = np.asarray(proj_w, dtype=np.float32)
    proj_b = np.asarray(proj_b, dtype=np.float32)
    ls = float(np.asarray(logit_scale, dtype=np.float32).reshape(-1)[0])
    scale = float(np.exp(min(ls, float(np.log(100.0)))))

    b, n, c3 = qkv.shape
    assert (b, n, c3) == (4, 16384, 768)

    in_maps, sort_idx, with_bias = _prep_inputs(qkv, sim, proj_w, proj_b, scale)

    key = ("b" if with_bias else "nb")
    if key not in _cache:
        _cache[key] = _build(with_bias)
    nc = _cache[key]

    global _last_in_maps
    _last_in_maps = in_maps
    res = run_bass_kernel_spmd(nc, in_maps, core_ids=list(range(8)))

    outf = np.empty((4, 16384, 256), dtype=np.float32)
    for cx in range(8):
        bi, half = cx // 2, cx % 2
        perm = sort_idx[bi, 8192 * half:8192 * (half + 1)]
        y = np.asarray(res.results[cx]["outd"]).astype(np.float32).reshape(8192, 256)
        outf[bi][perm] = y
    return outf


if __name__ == "__main__":
    rng = np.random.default_rng(0)
    qkv = rng.standard_normal((4, 16384, 768), dtype=np.float32)
    sim = rng.standard_normal((4, 16384, 64), dtype=np.float32)
    pw = (rng.standard_normal((256, 256), dtype=np.float32) * 0.02)
    pb = np.zeros(256, dtype=np.float32)
    lsc = np.log(10.0 * np.ones((1, 1), dtype=np.float32))
    o = kernel(qkv=qkv, sim=sim, proj_w=pw, proj_b=pb, logit_scale=lsc)
    print("ran", o.shape, o.dtype)
